# revision 49
# baseline (speedup 1.0000x reference)
"""CPC loss kernel for Trainium2, batch-sharded across 8 NeuronCores.

Shapes (hardcoded per problem spec):
  z, c: [2048, 64, 128] f32;  mask, neg_map: [128, 64] int;  W: [128, 128] f32
  ln_weight/ln_bias: [128] f32.  Output: scalar f32.

Per-core plan (Bc = 8 batch elements), bf16 data path:
  - Host packs per-core bf16 row tables zf/cf [SEQ*Bc, 128], int32 meta
    (interleaved pos/neg gather indices + keep multiplier), wpack = [W'^T|I].
  - Two fused indirect DMAs gather the 2*L*Bc z rows (pos/neg interleaved per
    batch), a third gathers the L*Bc c rows — only ~0.77MB read per core.
  - LN stats via bn_stats (mean + n*var per 128-elem chunk in one DVE pass),
    combined with strided-AP small ops; rsqrt via bit-trick + Newton.
  - Normalize fused to ONE tensor_scalar per segment using per-partition AP
    scalars: zln = z*rstd - mu*rstd.  Split across GpSimd (h0) / DVE (h1).
  - PE transposes (identity matmul) for ct and zln; E = W'^T @ ct^T and the
    per-batch MT = E_b^T @ zt_b on the PE in bf16.
  - diag(MT) pre-exp via one fused scalar_tensor_tensor (mult identity,
    accum_out) per batch on DVE; softmax denominator via Exp activation
    accum_out on ACT.
  - Device outputs raw [diag | den] [128, 2*Bc] f32; host does
    log(exp(diag)/den + 1e-3) and the mean in float64.

ln_weight folds into W on the host; ln_bias cancels in the softmax.  The keep
multiplier folds into rstd.  No max-subtraction needed: |logits| < ~70.
"""

import numpy as np

SEQ, B, L, ZD, CD = 2048, 64, 128, 128, 128
NCORES = 8
BC = B // NCORES  # 8
NSEG = 2 * BC  # 16 LN segments per core (interleaved pos/neg)
LN_EPS = 1e-5
USE_RANK1 = False
ACT_NORM_CHUNKS = 1  # how many 4-seg chunks normalize on ACT (0-4)
INTERLEAVE_NORM = False  # emit DVE norm chunks inside the pair loop
ZT_ON_A = 2   # pairs with PSUM->SBUF zt copy on ACT
DEN_V = 4     # tail batches whose den reduces on DVE

_cached = None


def _build_program():
    import concourse.bacc as bacc
    import concourse.tile as tile
    from concourse import bass, mybir

    f32 = mybir.dt.float32
    bf16 = mybir.dt.bfloat16
    i32 = mybir.dt.int32
    AF = mybir.ActivationFunctionType
    ALU = mybir.AluOpType
    AX = mybir.AxisListType

    nc = bacc.Bacc(
        "TRN2",
        target_bir_lowering=False,
        debug=False,
        enable_asserts=True,
        num_devices=NCORES,
    )

    zf_d = nc.dram_tensor("zf", [SEQ * BC, ZD], bf16, kind="ExternalInput")
    cf_d = nc.dram_tensor("cf", [SEQ * BC, CD], bf16, kind="ExternalInput")
    # meta: [:,0:16] interleaved pos/neg idx, [:,16:24] keep, [:,24:32] pos idx
    meta_d = nc.dram_tensor("meta", [L, 4 * BC], i32, kind="ExternalInput")
    # wpack: [:,0:128] = W'^T, [:,128:256] = identity (bf16)
    wpack_d = nc.dram_tensor("wpack", [128, 256], bf16, kind="ExternalInput")
    # out: [:,0:8] = diag (pre-exp), [:,8:16] = den
    out_d = nc.dram_tensor("out", [128, NSEG], f32, kind="ExternalOutput")

    with tile.TileContext(nc) as tc:
        with (
            tc.tile_pool(name="singles", bufs=1) as singles,
            tc.tile_pool(name="scratch", bufs=3) as scratch,
            tc.tile_pool(name="pwide", bufs=2, space="PSUM") as pwide,
            tc.tile_pool(name="pzt", bufs=2, space="PSUM") as ppzt,
            tc.tile_pool(name="pmt", bufs=4, space="PSUM") as ppmt,
            tc.tile_pool(name="paux", bufs=1, space="PSUM") as ppaux,
        ):
            # ---- ACT exp table preload (no input dependency) ----
            junk = singles.tile([1, 1], f32)
            nc.vector.memset(junk[:], 1.0)
            nc.scalar.activation(junk[:], junk[:], AF.Exp)
            ones1 = singles.tile([128, 1], bf16)
            nc.vector.memset(ones1[:], 1.0)

            # ---- small inputs: gathers depend only on meta ----
            meta_sb = singles.tile([L, 4 * BC], i32)
            nc.sync.dma_start(meta_sb[:], meta_d.ap())
            wpack_sb = singles.tile([128, 256], bf16)
            nc.sync.dma_start(wpack_sb[:], wpack_d.ap())
            wt_sb = wpack_sb[:, 0:128]
            ident_b = wpack_sb[:, 128:256]

            zidx = meta_sb[:, 0:NSEG]
            pos_idx = meta_sb[:, 3 * BC : 4 * BC]
            keep = meta_sb[:, NSEG : NSEG + BC].bitcast(f32)

            # ---- gathers: z half 0 (critical DVE chain), c (E-side),
            # ---- z half 1 ----
            zall = singles.tile([128, NSEG * ZD], bf16)
            call = singles.tile([128, BC * CD], bf16)
            for h in range(2):
                nc.gpsimd.indirect_dma_start(
                    out=zall[:, h * BC * ZD : (h + 1) * BC * ZD],
                    out_offset=None,
                    in_=zf_d.ap(),
                    in_offset=bass.IndirectOffsetOnAxis(
                        ap=zidx[:, h * BC : (h + 1) * BC], axis=0
                    ),
                )
            nc.gpsimd.indirect_dma_start(
                out=call[:],
                out_offset=None,
                in_=cf_d.ap(),
                in_offset=bass.IndirectOffsetOnAxis(ap=pos_idx, axis=0),
            )

            # ---- c-side: PE-transpose ct per batch, batched E = W' @ ct^T --
            ctT_sb = singles.tile([CD, BC * L], bf16)
            for g in range(2):
                pct = pwide.tile([128, 512], bf16, tag="pw")
                for k in range(4):
                    b = g * 4 + k
                    nc.tensor.transpose(
                        out=pct[:, k * 128 : (k + 1) * 128],
                        in_=call[:, b * CD : (b + 1) * CD],
                        identity=ident_b,
                    )
                nc.scalar.copy(ctT_sb[:, g * 512 : (g + 1) * 512], pct[:])
            e_sb = singles.tile([ZD, BC * L], bf16)
            for g in range(2):
                pe = pwide.tile([128, 512], f32, tag="pw")
                nc.tensor.matmul(
                    out=pe[:],
                    lhsT=wt_sb,
                    rhs=ctT_sb[:, g * 512 : (g + 1) * 512],
                    start=True,
                    stop=True,
                )
                nc.scalar.copy(e_sb[:, g * 512 : (g + 1) * 512], pe[:])
            # S[j,b] = sum_z E[z,(b,j)] — feeds the rank-1 mu-correction
            S_sb = singles.tile([1, BC * L], bf16)
            if USE_RANK1:
                for g in range(2):
                    pS = ppaux.tile([1, 512], f32, tag="ps")
                    nc.tensor.matmul(
                        out=pS[:],
                        lhsT=ones1[:],
                        rhs=e_sb[:, g * 512 : (g + 1) * 512],
                        start=True,
                        stop=True,
                    )
                    nc.scalar.copy(S_sb[:, g * 512 : (g + 1) * 512], pS[:])

            # ---- layernorm stats: bn_stats per 4-segment chunk ----
            # stats[:, c*24:(c+1)*24] = per chunk: 4 x (cnt_e, m_e, cv_e,
            #                                          cnt_o, m_o, cv_o)
            # bn_stats: one op per segment -> (cnt_e, m_e, cv_e, cnt_o,
            # m_o, cv_o) over the even/odd element interleaves; combine
            # with strided small ops.
            z3 = zall[:].rearrange("p (s d) -> p s d", d=ZD)
            stats = singles.tile([128, NSEG * 6], f32)
            msum = singles.tile([128, NSEG], f32)
            d_t = singles.tile([128, NSEG], f32)
            dd4 = singles.tile([128, NSEG], f32)
            vv = singles.tile([128, NSEG], f32)
            y = singles.tile([128, NSEG], f32)
            t1 = singles.tile([128, NSEG], f32)
            mr = singles.tile([128, NSEG], f32)
            rstd_bf = singles.tile([128, NSEG], bf16)
            mr_bf = singles.tile([128, NSEG], bf16)
            zln = singles.tile([128, NSEG * ZD], bf16)
            zl3 = zln[:].rearrange("p (s d) -> p s d", d=ZD)

            # all 16 bn_stats first (both z halves land back to back), then
            # ONE full-width [128,16] chain pass, then the 4 norm chunks —
            # keeps the late pairs' normalize as early as possible
            for s in range(NSEG):
                nc.vector.bn_stats(
                    out=stats[:, s * 6 : (s + 1) * 6],
                    in_=zall[:, s * ZD : (s + 1) * ZD],
                )
            me = stats[:, 1 : 96 : 6]
            mo = stats[:, 4 : 96 : 6]
            cve = stats[:, 2 : 96 : 6]
            cvo = stats[:, 5 : 96 : 6]
            nc.vector.tensor_tensor(out=msum[:], in0=me, in1=mo, op=ALU.add)
            nc.vector.tensor_tensor(
                out=d_t[:], in0=me, in1=mo, op=ALU.subtract
            )
            # dd4 = (d*0.25)*d = ((m_e-m_o)/2)^2
            nc.vector.scalar_tensor_tensor(
                out=dd4[:], in0=d_t[:], scalar=0.25,
                in1=d_t[:], op0=ALU.mult, op1=ALU.mult,
            )
            # vv = (cve+cvo)/ZD + eps + dd4 = var + eps
            nc.vector.tensor_tensor(out=vv[:], in0=cve, in1=cvo, op=ALU.add)
            nc.vector.tensor_scalar(
                out=vv[:], in0=vv[:], scalar1=1.0 / ZD,
                scalar2=LN_EPS, op0=ALU.mult, op1=ALU.add,
            )
            nc.vector.tensor_tensor(
                out=vv[:], in0=vv[:], in1=dd4[:], op=ALU.add
            )
            # rstd = rsqrt(vv) via bit trick + 1 Newton iteration (the
            # ~0.2% residual scale error washes out in the mean loss)
            nc.vector.tensor_scalar(
                out=y[:].bitcast(i32), in0=vv[:].bitcast(i32),
                scalar1=1, scalar2=None, op0=ALU.arith_shift_right,
            )
            nc.vector.tensor_scalar(
                out=y[:].bitcast(i32), in0=y[:].bitcast(i32),
                scalar1=-1, scalar2=0x5F3759DF, op0=ALU.mult, op1=ALU.add,
            )
            for _ in range(1):
                nc.vector.tensor_mul(t1[:], y[:], y[:])
                nc.vector.tensor_mul(t1[:], t1[:], vv[:])
                nc.vector.tensor_scalar(
                    out=t1[:], in0=t1[:], scalar1=-0.5, scalar2=1.5,
                    op0=ALU.mult, op1=ALU.add,
                )
                nc.vector.tensor_mul(y[:], y[:], t1[:])
            rstd = y
            # zero out masked negative rows (odd segments)
            nc.vector.tensor_mul(rstd[:, 1::2], rstd[:, 1::2], keep[:])
            # mr = mu*rstd = (msum*0.5)*rstd; bf16 copies for the wide
            # normalize ops
            nc.vector.scalar_tensor_tensor(
                out=mr[:], in0=msum[:], scalar=-0.5,
                in1=rstd[:], op0=ALU.mult, op1=ALU.mult,
            )
            nc.vector.tensor_copy(rstd_bf[:], rstd[:])
            nmrT_flat = singles.tile([1, NSEG * 128], bf16)
            if USE_RANK1:
                # nmr = -mu*rstd as bf16; flatten mr_bf [128(pos),16(seg)]
                # row-major onto partition 0 via DMA (the only engine that
                # crosses partitions): nmr[pos,s] lands at column pos*16+s,
                # so the rank-1 rhs for segment s is the stride-16 AP at s.
                nc.vector.tensor_copy(mr_bf[:], mr[:])
                nc.sync.dma_start(
                    nmrT_flat[0:1, :].rearrange("a (p s) -> a p s", s=NSEG),
                    mr_bf[:],
                )
            else:
                nc.vector.tensor_copy(mr_bf[:], mr[:])
            # normalize: zs = z*rstd (mu via the rank-1 fold or a second
            # broadcast subtract), all on DVE (GpSimd contends for SBUF
            # ports with DVE)
            # late chunks on ACT: zln = Ident(z*rstd + nmr) with
            # per-partition scale/bias APs — starts as soon as the chain
            # lands, freeing DVE for its tail work
            for s in range(4 * (4 - ACT_NORM_CHUNKS), 16):
                nc.scalar.activation(
                    zln[:, s * ZD : (s + 1) * ZD],
                    zall[:, s * ZD : (s + 1) * ZD],
                    AF.Identity,
                    bias=mr[:, s : s + 1],
                    scale=rstd[:, s : s + 1],
                )


            # ---- per-pair: DVE norm chunk emitted right before its
            # ---- consumer transposes (keeps Tile's coarsened semaphores
            # ---- from making pair p wait on later chunks), MT matmul,
            # ---- diag (DVE), exp with den accumulation (ACT) ----
            outv = singles.tile([128, NSEG], f32)  # [diag | den]
            if not INTERLEAVE_NORM:
                for ch in range(4 - ACT_NORM_CHUNKS):
                    ssl = slice(4 * ch, 4 * ch + 4)
                    rstd_bc = rstd_bf[:, ssl].unsqueeze(-1).to_broadcast(
                        [128, 4, ZD]
                    )
                    nc.vector.tensor_tensor(
                        out=zl3[:, ssl, :], in0=z3[:, ssl, :], in1=rstd_bc,
                        op=ALU.mult,
                    )
                    mr_bc = mr_bf[:, ssl].unsqueeze(-1).to_broadcast(
                        [128, 4, ZD]
                    )
                    nc.vector.tensor_tensor(
                        out=zl3[:, ssl, :], in0=zl3[:, ssl, :],
                        in1=mr_bc, op=ALU.add,
                    )
            for p in range(BC // 2):
                if INTERLEAVE_NORM and p < 4 - ACT_NORM_CHUNKS:
                    ssl = slice(4 * p, 4 * p + 4)
                    rstd_bc = rstd_bf[:, ssl].unsqueeze(-1).to_broadcast(
                        [128, 4, ZD]
                    )
                    nc.vector.tensor_tensor(
                        out=zl3[:, ssl, :], in0=z3[:, ssl, :], in1=rstd_bc,
                        op=ALU.mult,
                    )
                    if not USE_RANK1:
                        mr_bc = mr_bf[:, ssl].unsqueeze(-1).to_broadcast(
                            [128, 4, ZD]
                        )
                        nc.vector.tensor_tensor(
                            out=zl3[:, ssl, :], in0=zl3[:, ssl, :],
                            in1=mr_bc, op=ALU.add,
                        )
                pzt = ppzt.tile([128, 512], bf16, tag="pzt")
                for i in range(4):
                    s = 4 * p + i
                    nc.tensor.transpose(
                        out=pzt[:, i * 128 : (i + 1) * 128],
                        in_=zln[:, s * ZD : (s + 1) * ZD],
                        identity=ident_b,
                    )
                zt_sb = scratch.tile([128, 512], bf16, tag="zt")
                # early pairs copied on ACT (free then); late pairs on DVE
                # (free then) so the ACT tail is pure exp
                if p < ZT_ON_A:
                    nc.scalar.copy(zt_sb[:], pzt[:])
                else:
                    nc.vector.tensor_copy(zt_sb[:], pzt[:])
                for k in range(2):
                    b = 2 * p + k
                    pmt_b = ppmt.tile([128, 256], f32, tag="pmt")
                    nc.tensor.matmul(
                        out=pmt_b[:],
                        lhsT=e_sb[:, b * L : (b + 1) * L],
                        rhs=zt_sb[:, k * 256 : (k + 1) * 256],
                        start=True,
                        stop=not USE_RANK1,
                        skip_group_check=True,
                    )
                    if USE_RANK1:
                        # rank-1 mu-correction: MT += S_b (x) (-mu*rstd)
                        for half in range(2):
                            s = 2 * b + half
                            nc.tensor.matmul(
                                out=pmt_b[
                                    :, half * 128 : (half + 1) * 128
                                ],
                                lhsT=S_sb[:, b * L : (b + 1) * L],
                                rhs=nmrT_flat[:, s :: NSEG],
                                start=False,
                                stop=True,
                                skip_group_check=True,
                            )
                    # diag (pre-exp) via fused mult-by-identity + accum
                    numt = scratch.tile([128, 128], f32, tag="numt")
                    nc.vector.scalar_tensor_tensor(
                        out=numt[:], in0=pmt_b[:, 0:128],
                        scalar=1.0, in1=ident_b, op0=ALU.mult, op1=ALU.mult,
                        accum_out=outv[:, b : b + 1],
                    )
                    expt = scratch.tile([128, 256], bf16, tag="expt")
                    if b < BC - DEN_V:
                        nc.scalar.activation(
                            expt[:], pmt_b[:], AF.Exp,
                            accum_out=outv[:, BC + b : BC + b + 1],
                        )
                    else:
                        # tail batches: den reduced on DVE (idle by then)
                        # so ACT does nothing but the exp itself
                        nc.scalar.activation(expt[:], pmt_b[:], AF.Exp)
                        nc.vector.tensor_reduce(
                            out=outv[:, BC + b : BC + b + 1],
                            in_=expt[:],
                            axis=AX.X,
                            op=ALU.add,
                        )
            nc.sync.dma_start(out_d.ap(), outv[:])

    nc.compile()
    return nc


def _prep_in_maps(z, c, mask, neg_map, W, ln_weight):
    import ml_dtypes

    bf = ml_dtypes.bfloat16
    z = np.asarray(z, dtype=np.float32)
    c = np.asarray(c, dtype=np.float32)
    mask = np.asarray(mask).astype(np.int64)
    neg_map = np.asarray(neg_map).astype(np.int64)
    W = np.asarray(W, dtype=np.float32)
    ln_weight = np.asarray(ln_weight, dtype=np.float32)

    wt = (ln_weight[:, None] * W).T  # [c, z] = W'[z, c]
    wpack = np.ascontiguousarray(
        np.concatenate([wt, np.eye(128, dtype=np.float32)], axis=1)
    ).astype(bf)
    boff = np.arange(BC, dtype=np.int64)[None, :]
    in_maps = []
    for i in range(NCORES):
        bsl = slice(i * BC, (i + 1) * BC)
        zf = np.ascontiguousarray(z[:, bsl, :]).reshape(SEQ * BC, ZD).astype(bf)
        cf = np.ascontiguousarray(c[:, bsl, :]).reshape(SEQ * BC, CD).astype(bf)
        m = mask[:, bsl]
        n = neg_map[:, bsl]
        pos_idx = (m * BC + boff).astype(np.int32)
        neg_idx = (n * BC + boff).astype(np.int32)
        zidx = np.empty((L, NSEG), dtype=np.int32)
        zidx[:, 0::2] = pos_idx
        zidx[:, 1::2] = neg_idx
        hit = (n[:, None, :] == m[None, :, :]).any(axis=1)  # [L, BC]
        keep = (~hit).astype(np.float32)
        meta = np.concatenate(
            [zidx, keep.view(np.int32), pos_idx], axis=1
        ).astype(np.int32)
        in_maps.append({"zf": zf, "cf": cf, "meta": meta, "wpack": wpack})
    return in_maps


def _combine(results):
    total = np.float64(0.0)
    for r in results:
        o = np.asarray(r["out"], dtype=np.float64)
        diag, den = o[:, 0:BC], o[:, BC : 2 * BC]
        total += np.log(np.exp(diag) / den + 1e-3).sum()
    return np.float32(-(total / (L * B)))


def kernel(z, c, mask, neg_map, W, ln_weight, ln_bias):
    from concourse import bass_utils

    global _cached
    if _cached is None:
        _cached = _build_program()
    nc = _cached

    in_maps = _prep_in_maps(z, c, mask, neg_map, W, ln_weight)
    res = bass_utils.run_bass_kernel_spmd(
        nc, in_maps, core_ids=list(range(NCORES))
    )
    return _combine(res.results)
